# revision 14
# baseline (speedup 1.0000x reference)
"""EnergyTransformer TRN2 Bass kernel.

The reference performs 12 steps of Armijo/BB gradient descent on an energy
E(x) = E_att(LN(x)) + E_hopfield(LN(x)).  Algebraically the reference's
trajectory freezes after step 0: it assigns prev_x = x AFTER the update, so
at every step t>=1, s = x - prev_x == 0 exactly, hence ss = sy = 0, the BB
step lr0 = 0/max(0,1e-8) = 0.0, and chosen = lr0 * gamma^k = 0.0, leaving x
bit-exactly unchanged.  Step 0 uses lr0 = ALPHA = 1.0 and its Armijo
backtracking accepts the full step (energy margins are ~1e4..1e5, far beyond
fp32 noise; verified in fp64 + against the jax reference).  Therefore:

    output = x - grad(E)(x)

computed as a single fused forward+backward pass, data-parallel over the
batch (B=8) across 8 NeuronCores (one element per core, no collectives).

Backward math (per batch element, N=196 tokens, D=768, H=12 heads, Y=64,
M=3072 memories):
    ghat = (x - mu) / sqrt(var + eps)            (token LayerNorm, biased var)
    g    = gamma*ghat + delta
    K = g @ Wk^T, Q = g @ Wq^T                   (Wk,Wq: [H*Y, D])
    S_h = beta * Q_h K_h^T ; P_h = softmax_k(S_h)
    Hr  = relu(g @ Xi^T)                         (Xi: [M, D])
    dE/dg = -[ (P_h^T Q_h) Wk_h + (P_h K_h) Wq_h ]_h - Hr @ Xi
    grad = inv * (dghat - mean(dghat) - ghat * mean(dghat*ghat))
    out  = x - grad

Implementation notes:
  - gamma folded into weights host-side; delta enters as bias vectors.
  - beta folded into K (host scales Wk-lhsT part and bk by beta; wqr is
    divided by beta to compensate the dQ path).
  - softmax without max-subtraction: scores are O(+-3) after beta, so
    exp(S - ln(N)) is safe; denominators come from the exp's accum_out.
  - P^T is NOT transposed on the PE: S^T is recomputed by matmul from
    per-head [65, N] K/Q tiles whose 65th row carries (ones | -ln(den)),
    so exp(S^T - c - ln den) IS the normalized P^T.  The dK path uses the
    unnormalized exp(S) with 1/den folded into the per-head Q' slices.
  - dG matmul's rhs weights carry an extra summed column so sum_d(dG)
    falls out of the PSUM for free; LN backward is fused into
    scalar_tensor_tensor/activation ops:
        out = x*(1 + inv*m2n) + ACT(u; scale=inv, bias=(negmu*m2n - mean_u)*inv)
        m2n = -mean(u*ghat)*inv
  - All DRAM traffic is packed partition-major so each logical stream is
    1-2 dma_start instructions (per-dma_start sequencer cost ~2.6us).
"""

import numpy as np

import concourse.bass as bass
import concourse.mybir as mybir
import concourse.tile as tile
from concourse import bacc
from concourse import bass_utils

# Problem dims (hardcoded per contest contract).
B, N, D, H, Y, M = 8, 196, 768, 12, 64, 3072
HY = H * Y          # 768
NCORES = 8
LN_EPS = 1e-5
BETA = 1.0 / float(np.sqrt(Y))
C_LSE = float(np.log(N))    # constant shift inside both exps

NT = 2              # n tiles: 128 + 68
NSZ = [128, N - 128]
NOFF = [0, 128]
DT_ = D // 128      # 6
HT_ = HY // 128     # 6
MT_ = M // 128      # 24
DW = D + 1          # backward weight row width (sum column appended)
CH = [(0, 512), (512, 257)]   # free-dim chunks of DW for backward matmuls
# fp8 DoubleRow chunks: moving free = 2*cw <= 512
CH8 = [(0, 256), (256, 256), (512, 129), (641, 128)]

# fp8 weight blob column offsets (e4m3)
OFF_WKT = 0
OFF_WQT = DT_ * HY              # 4608
OFF_WKQR = 2 * DT_ * HY         # 9216
W8BLOB = OFF_WKQR + 2 * HT_ * DW   # 18444
XTBLOB = MT_ * D                # 18432 (fwd layout, no sum col)
XRBLOB = MT_ * DW               # 18456

# fp8 scale plumbing: products of the attention dG fp8 operands carry
# S1*S2 = SG; the Hopfield fwd weights are pre-scaled by SG so all pg
# contributions share one scale, compensated in the LN-backward constants.
SG = 64.0           # global pg scale
S1 = 8.0            # dkt/dqt quantization scale
S2 = SG / S1        # wkr/wqr host scale
SK = 128.0          # wkt (beta-folded K weights) host scale
SQ = 128.0          # wqt host scale

MODE = "bf16"

# Diagnostic: load the xi streams once instead of per rep.
HOIST = True

# Timing: repeat the whole compute body REPS times in one program.
REPS = 1


def _nonce_cols():
    # The jax/neuronx NEFF cache keys on the HLO module, which does NOT
    # capture the bass program; a nonce input whose shape depends on the
    # kernel source (and REPS) makes stale-cache reuse impossible.
    import zlib
    h = zlib.crc32(open(__file__, "rb").read()) ^ (REPS * 7919)
    return 1 + (h % 509)

_CACHE = {}


def _np_mmdt():
    if MODE == "f32":
        return np.float32
    import ml_dtypes
    return ml_dtypes.bfloat16


def _pin_act_table():
    """Bias the act-table insertion pass to a single set that contains every
    activation function this kernel uses (Exp, Ln, Square, Copy, Identity,
    Relu), so no mid-kernel LoadActFuncSet swaps (1283 ns each) are emitted.
    Reordering is semantically neutral: the pass only picks which valid
    table id to load."""
    import concourse.bacc as _bacc

    real = _bacc.get_activation_tables
    if getattr(_bacc.get_activation_tables, "_pinned", False):
        return

    def pinned(arch):
        # Keep names/positions (act_func_set_id is the canonical index into
        # act_info.json) but hide every other set from the chooser, so all
        # loads resolve to the one superset table.
        tabs = real(arch)
        pref = "natural_log_exp_and_others"
        if pref not in tabs:
            return tabs
        return {k: (v if k == pref else type(v)()) for k, v in tabs.items()}

    pinned._pinned = True
    _bacc.get_activation_tables = pinned


def build_program():
    from concourse.masks import make_identity
    from concourse.mybir import dt

    _pin_act_table()

    F32 = dt.float32
    MMDT = F32 if MODE == "f32" else dt.bfloat16
    AF = mybir.ActivationFunctionType
    ALU = mybir.AluOpType
    AX = mybir.AxisListType

    nc = bacc.Bacc("TRN2", target_bir_lowering=False, debug=False,
                   num_devices=NCORES)

    F8 = dt.float8e4
    x_d = nc.dram_tensor("x", [128, 2 * D], F32, kind="ExternalInput").ap()
    w_d = nc.dram_tensor("wblob", [128, W8BLOB], F8,
                         kind="ExternalInput").ap()
    xit_d = nc.dram_tensor("xitb", [128, XTBLOB], MMDT,
                           kind="ExternalInput").ap()
    xir_d = nc.dram_tensor("xirb", [128, XRBLOB], MMDT,
                           kind="ExternalInput").ap()
    bias_d = nc.dram_tensor("biasb", [128, 36], F32, kind="ExternalInput").ap()
    nonce_d = nc.dram_tensor("nonce", [1, _nonce_cols()], F32,
                             kind="ExternalInput").ap()
    out_d = nc.dram_tensor("out", [128, 2 * D], F32, kind="ExternalOutput").ap()

    with tile.TileContext(nc) as tc:
        with (
            tc.tile_pool(name="persist", bufs=1) as pp,
            tc.tile_pool(name="stats", bufs=4) as sp,
            tc.tile_pool(name="scratch", bufs=2) as scp,
            tc.tile_pool(name="rot", bufs=8) as rp,
        ):
            ident = pp.tile([128, 128], F32, name="ident", tag="ident")
            make_identity(nc, ident[:])
            if MMDT != F32:
                identb = pp.tile([128, 128], MMDT, name="identb", tag="identb")
                nc.vector.tensor_copy(identb[:], ident[:])
            else:
                identb = ident

            eps_t = pp.tile([128, 1], F32, name="eps_t", tag="eps_t")
            nc.gpsimd.memset(eps_t[:], float(LN_EPS))
            nclse = pp.tile([128, 1], F32, name="nclse", tag="nclse")
            nc.gpsimd.memset(nclse[:], -C_LSE)

            bias_t = pp.tile([128, 36], F32, name="bias_t", tag="bias_t")
            nc.sync.dma_start(bias_t[:], bias_d)
            nonce_t = pp.tile([1, 1], F32, name="nonce_t", tag="nonce_t")
            nc.gpsimd.dma_start(nonce_t[:], nonce_d[0:1, 0:1])

            def bh_col(mt):
                return bias_t[:, 12 + mt:13 + mt]

            # resident weights: one blob, one DMA
            wall = pp.tile([128, WBLOB], MMDT, name="wall", tag="wall")
            nc.sync.dma_start(wall[:], w_d)

            def wkt_s(j, c0, c1):
                return wall[:, OFF_WKT + j * HY + c0:OFF_WKT + j * HY + c1]

            def wqt_s(j, c0, c1):
                return wall[:, OFF_WQT + j * HY + c0:OFF_WQT + j * HY + c1]

            def wkr_s(i, c0, c1):
                return wall[:, OFF_WKR + i * DW + c0:OFF_WKR + i * DW + c1]

            def wqr_s(i, c0, c1):
                return wall[:, OFF_WQR + i * DW + c0:OFF_WQR + i * DW + c1]

            # streamed per rep: xi in lhsT layout (fwd) and row layout (bwd)
            xit_t = pp.tile([128, XTBLOB], MMDT, name="xit_t", tag="xit_t")
            xir_t = pp.tile([128, XRBLOB], MMDT, name="xir_t", tag="xir_t")
            o_pack = pp.tile([128, 2 * D], F32, name="o_pack", tag="o_pack")
            nc.gpsimd.memset(o_pack[64:, D:], 0.0)

            # per-head K/Q tiles: rows 0:64 data, row 64 = (ones | -ln den)
            kth = []
            qth = []
            for h in range(H):
                k_ = pp.tile([65, N], MMDT, name=f"kth{h}", tag=f"kth{h}")
                q_ = pp.tile([65, N], MMDT, name=f"qth{h}", tag=f"qth{h}")
                nc.gpsimd.memset(k_[64:65, :], 1.0)
                kth.append(k_)
                qth.append(q_)

            if HOIST:
                nc.sync.dma_start(xit_t[:, :XTBLOB // 2],
                                  xit_d[:, :XTBLOB // 2])
                nc.gpsimd.dma_start(xit_t[:, XTBLOB // 2:],
                                    xit_d[:, XTBLOB // 2:])
                nc.gpsimd.dma_start(xir_t[:, :XRBLOB // 2],
                                    xir_d[:, :XRBLOB // 2])
                nc.gpsimd.dma_start(xir_t[:, XRBLOB // 2:],
                                    xir_d[:, XRBLOB // 2:])

            for _rep in range(REPS):
                g = _rep % 2
                xt = pp.tile([128, 2 * D], F32, name=f"xt{g}", tag=f"xt{g}")
                nc.sync.dma_start(xt[:], x_d)
                if not HOIST:
                    nc.sync.dma_start(xit_t[:, :XTBLOB // 2],
                                      xit_d[:, :XTBLOB // 2])
                    nc.gpsimd.dma_start(xit_t[:, XTBLOB // 2:],
                                        xit_d[:, XTBLOB // 2:])
                    nc.gpsimd.dma_start(xir_t[:, :XRBLOB // 2],
                                        xir_d[:, :XRBLOB // 2])
                    nc.gpsimd.dma_start(xir_t[:, XRBLOB // 2:],
                                        xir_d[:, XRBLOB // 2:])

                with (
                    tc.tile_pool(name="psmA", bufs=5, space="PSUM") as psm,
                ):
                    # ---------------- LayerNorm forward ----------------
                    x_t = []
                    ghat = []
                    inv = []
                    negmu_l = []
                    for ns in range(NT):
                        P = NSZ[ns]
                        xs = xt[:P, ns * D:(ns + 1) * D]
                        gh = pp.tile([P, D], F32, name=f"ghat{ns}_{g}", tag=f"ghat{ns}_{g}")
                        iv = pp.tile([P, 1], F32, name=f"inv{ns}_{g}", tag=f"inv{ns}_{g}")
                        negmu = pp.tile([P, 1], F32, name=f"negmu{ns}_{g}",
                                        tag=f"negmu{ns}_{g}")
                        negsum = sp.tile([P, 1], F32, name="negsum", tag="negsum")
                        ssum = sp.tile([P, 1], F32, name="ssum", tag="ssum")
                        std = sp.tile([P, 1], F32, name="std", tag="std")
                        scr = scp.tile([128, D], F32, name="scr", tag="scr")
                        nc.vector.tensor_reduce(negsum[:], xs, AX.X, ALU.add,
                                                negate=True)
                        nc.vector.tensor_scalar_mul(negmu[:], negsum[:], 1.0 / D)
                        nc.scalar.activation(scr[:P, :], xs, AF.Square,
                                             bias=negmu[:], scale=1.0,
                                             accum_out=ssum[:])
                        nc.scalar.activation(std[:], ssum[:], AF.Ln,
                                             bias=eps_t[:P, :], scale=1.0 / D)
                        nc.scalar.activation(iv[:], std[:], AF.Exp,
                                             scale=-0.5)
                        nc.vector.tensor_scalar(gh[:], xs, negmu[:], iv[:],
                                                ALU.add, ALU.mult)
                        x_t.append(xs)
                        ghat.append(gh)
                        inv.append(iv)
                        negmu_l.append(negmu)

                    # ---------------- transpose ghat -> ghatT [d, n] ----
                    ghatT = []
                    with tc.tile_pool(name="psgt", bufs=2,
                                      space="PSUM") as psgt:
                        for j in range(DT_):
                            gt = pp.tile([128, N], MMDT, name=f"ghatT{j}_{g}",
                                         tag=f"ghatT{j}_{g}")
                            ps = psgt.tile([128, N], F32, name="ps_gt",
                                           tag="ps_gt")
                            for ns in range(NT):
                                P = NSZ[ns]
                                nc.tensor.transpose(
                                    ps[:, NOFF[ns]:NOFF[ns] + P],
                                    ghat[ns][:, j * 128:(j + 1) * 128],
                                    ident[:P, :P])
                            nc.vector.tensor_copy(gt[:], ps[:])
                            ghatT.append(gt)

                    # ------- K^T, Q^T into per-head [65, N] tiles -------
                    for wt, bofs, dst in ((wkt_s, 0, kth), (wqt_s, 6, qth)):
                        for i in range(HT_):
                            ps = psm.tile([128, N], F32, name="psmm", tag="psmm")
                            for j in range(DT_):
                                nc.tensor.matmul(
                                    ps[:], wt(j, i * 128, (i + 1) * 128),
                                    ghatT[j][:], start=(j == 0),
                                    stop=(j == DT_ - 1))
                            nc.vector.tensor_scalar_add(
                                dst[2 * i][0:64, :], ps[0:64, :],
                                bias_t[0:64, bofs + i:bofs + i + 1])
                            nc.vector.tensor_scalar_add(
                                dst[2 * i + 1][0:64, :], ps[64:128, :],
                                bias_t[64:128, bofs + i:bofs + i + 1])

                    # ---------------- K', Q'  [n, hy] (transposes) ------
                    kp = []
                    qp = []
                    with tc.tile_pool(name="pskq", bufs=2,
                                      space="PSUM") as pskq:
                        for src, dst, nm in ((kth, kp, "kp"), (qth, qp, "qp")):
                            for ns in range(NT):
                                P = NSZ[ns]
                                o = pp.tile([P, HY], MMDT, name=f"{nm}{ns}",
                                            tag=f"{nm}{ns}")
                                for i in range(HT_):
                                    ps = pskq.tile([128, 128], MMDT,
                                                   name="pstr2", tag="pstr2")
                                    for k in range(2):
                                        nc.tensor.transpose(
                                            ps[:P, k * 64:(k + 1) * 64],
                                            src[2 * i + k][0:64,
                                                           NOFF[ns]:NOFF[ns] + P],
                                            identb[:64, :64])
                                    nc.vector.tensor_copy(
                                        o[:, i * 128:(i + 1) * 128],
                                        ps[:P, :])
                                dst.append(o)

                    # ------------- pass A: scores exp + Hopfield fwd ----
                    hrT = []
                    for mt in range(MT_):
                        hr = pp.tile([128, N], MMDT, name=f"hrT{mt}",
                                     tag=f"hrT{mt}")
                        hrT.append(hr)

                    den = []
                    for ns in range(NT):
                        dn = pp.tile([NSZ[ns], H], F32, name=f"den{ns}",
                                     tag=f"den{ns}")
                        den.append(dn)

                    e_h = [[None] * NT for _ in range(H)]

                    def hop_fwd(mt):
                        ps = psm.tile([128, N], F32, name="psmm", tag="psmm")
                        for j in range(DT_):
                            nc.tensor.matmul(
                                ps[:],
                                xit_t[:, mt * D + j * 128:mt * D + (j + 1) * 128],
                                ghatT[j][:], start=(j == 0),
                                stop=(j == DT_ - 1))
                        nc.vector.tensor_scalar(hrT[mt][:], ps[:],
                                                bh_col(mt), 0.0, ALU.add,
                                                ALU.max)

                    for h in range(H):
                        for ns in range(NT):
                            P = NSZ[ns]
                            ps = psm.tile([128, N], F32, name="psmm", tag="psmm")
                            nc.tensor.matmul(
                                ps[:P, :],
                                qth[h][0:64, NOFF[ns]:NOFF[ns] + P],
                                kth[h][0:64, :],
                                start=True, stop=True)
                            e = pp.tile([P, N], MMDT, name=f"e{h}_{ns}",
                                        tag=f"e{h}_{ns}")
                            nc.scalar.activation(e[:], ps[:P, :], AF.Exp,
                                                 bias=nclse[:P, :], scale=1.0,
                                                 accum_out=den[ns][:, h:h + 1])
                            e_h[h][ns] = e
                        hop_fwd(2 * h)
                        hop_fwd(2 * h + 1)

                # ---- normalization constants: 1/den, -ln den rows --
                invd = []
                lnlb = []
                for ns in range(NT):
                    P = NSZ[ns]
                    lnf = sp.tile([P, H], F32, name="lnf", tag="lnf")
                    nc.scalar.activation(lnf[:], den[ns][:], AF.Ln)
                    iv = sp.tile([P, H], F32, name="invd", tag="invd")
                    nc.scalar.activation(iv[:], lnf[:], AF.Exp, scale=-1.0)
                    invd.append(iv)
                    lb = sp.tile([P, H], MMDT, name="lnlb", tag="lnlb")
                    nc.vector.tensor_copy(lb[:], lnf[:])
                    lnlb.append(lb)
                with tc.tile_pool(name="psrow", bufs=2,
                                  space="PSUM") as psrow:
                    for h in range(H):
                        ps = psrow.tile([1, N], MMDT, name="ps_row",
                                        tag="ps_row")
                        for ns in range(NT):
                            P = NSZ[ns]
                            nc.tensor.transpose(
                                ps[0:1, NOFF[ns]:NOFF[ns] + P],
                                lnlb[ns][:P, h:h + 1], identb[:P, :P])
                        nc.vector.tensor_scalar_mul(qth[h][64:65, :],
                                                    ps[0:1, :], -1.0)
                # fold 1/den into Q' per head (dK path) — on the idle Pool
                # engine so DVE stays free for the backward stream; h-major
                # order so dK(i) unblocks after 2*(i+1) pairs of ops
                for h in range(H):
                    for ns in range(NT):
                        nc.gpsimd.tensor_scalar_mul(
                            qp[ns][:, h * 64:(h + 1) * 64],
                            qp[ns][:, h * 64:(h + 1) * 64],
                            invd[ns][:, h:h + 1])

                dkt_t = []
                dqt_t = []
                for i in range(HT_):
                    dk = pp.tile([128, N], MMDT, name=f"dkt{i}", tag=f"dkt{i}")
                    dq = pp.tile([128, N], MMDT, name=f"dqt{i}", tag=f"dqt{i}")
                    dkt_t.append(dk)
                    dqt_t.append(dq)
                with (
                    tc.tile_pool(name="psdg", bufs=1, space="PSUM") as psdg,
                    tc.tile_pool(name="psmB", bufs=2, space="PSUM") as psmB,
                ):
                    pg = []
                    for ns in range(NT):
                        row = []
                        for ci, (_, cw) in enumerate(CH):
                            t = psdg.tile([NSZ[ns], cw], mybir.dt.float32,
                                          name=f"pg{ns}_{ci}", tag=f"pg{ns}_{ci}")
                            row.append(t)
                        pg.append(row)
                    NBLK = 2 * HT_ + MT_
                    bi_c = [0] * NT

                    def dg_block(ns, kind, idx):
                        bi = bi_c[ns]
                        bi_c[ns] += 1
                        lhs = {"dkt": dkt_t, "dqt": dqt_t,
                               "hr": hrT}[kind][idx]
                        P = NSZ[ns]
                        for ci, (c0, cw) in enumerate(CH):
                            if kind == "dkt":
                                w = wkr_s(idx, c0, c0 + cw)
                            elif kind == "dqt":
                                w = wqr_s(idx, c0, c0 + cw)
                            else:
                                w = xir_t[:, idx * DW + c0:idx * DW + c0 + cw]
                            nc.tensor.matmul(pg[ns][ci][:],
                                             lhs[:, NOFF[ns]:NOFF[ns] + P],
                                             w,
                                             start=(bi == 0),
                                             stop=(bi == NBLK - 1))

                    def tail_ns(ns):
                        # LN backward + output for one row-tile:
                        #   out = x*(1 + inv*m2n)
                        #         + ACT(u; scale=inv, bias=(negmu*m2n-mean_u)*inv)
                        #   m2n = -mean(u*ghat)*inv
                        P = NSZ[ns]
                        m2a = sp.tile([P, 1], F32, name="m2a", tag="m2a")
                        m2b = sp.tile([P, 1], F32, name="m2b", tag="m2b")
                        m2n = sp.tile([P, 1], F32, name="m2n", tag="m2n")
                        a_ = sp.tile([P, 1], F32, name="a_", tag="a_")
                        bias1 = sp.tile([P, 1], F32, name="bias1", tag="bias1")
                        w1 = sp.tile([P, 1], F32, name="w1", tag="w1")
                        t1l = scp.tile([128, D], F32, name="t1", tag="t1")
                        scr = scp.tile([128, D], F32, name="scr", tag="scr")
                        nc.vector.scalar_tensor_tensor(
                            scr[:P, 0:512], pg[ns][0][:], 1.0,
                            ghat[ns][:, 0:512], ALU.mult, ALU.mult,
                            accum_out=m2a[:])
                        nc.vector.scalar_tensor_tensor(
                            scr[:P, 512:768], pg[ns][1][:, 0:256], 1.0,
                            ghat[ns][:, 512:768], ALU.mult, ALU.mult,
                            accum_out=m2b[:])
                        nc.vector.tensor_add(m2n[:], m2a[:], m2b[:])
                        nc.vector.tensor_scalar_mul(m2n[:], m2n[:], -1.0 / D)
                        nc.vector.tensor_mul(m2n[:], m2n[:], inv[ns][:])
                        nc.vector.tensor_mul(a_[:], inv[ns][:], m2n[:])
                        nc.vector.tensor_scalar_add(a_[:], a_[:], 1.0)
                        nc.vector.tensor_mul(w1[:], negmu_l[ns][:], m2n[:])
                        nc.vector.tensor_scalar(bias1[:],
                                                pg[ns][1][:, 256:257],
                                                -1.0 / D, None, ALU.mult)
                        nc.vector.tensor_add(bias1[:], bias1[:], w1[:])
                        nc.vector.tensor_mul(bias1[:], bias1[:], inv[ns][:])
                        nc.scalar.activation(t1l[:P, 0:512], pg[ns][0][:],
                                             AF.Identity, bias=bias1[:],
                                             scale=inv[ns][:])
                        nc.scalar.activation(t1l[:P, 512:768],
                                             pg[ns][1][:, 0:256],
                                             AF.Identity, bias=bias1[:],
                                             scale=inv[ns][:])
                        o = o_pack[:P, ns * D:(ns + 1) * D]
                        nc.vector.scalar_tensor_tensor(
                            o, x_t[ns], a_[:], t1l[:P, :], ALU.mult, ALU.add)
                        nc.sync.dma_start(out_d[:, ns * D:(ns + 1) * D],
                                          o_pack[:, ns * D:(ns + 1) * D])

                    if True:
                        # ns=0's 36 dG blocks run woven into the shared
                        # dS/dK/dQ stream; ns=1's run as one clean block after,
                        # covering ns=0's LN-backward tail on Act/DVE.
                        # Hopfield dG blocks depend only on hrT/xir (ready at
                        # end of pass A) — the first few fill the PE gap while
                        # Act/DVE compute den normalization; the rest keep the
                        # PE fed while Act runs the S^T exps.
                        for mt in range(4):
                            dg_block(0, "hr", mt)
                        for i in range(HT_):
                            ps_dk = psmB.tile([128, N], F32, name="ps_dk",
                                              tag="ps_dk", bufs=1)
                            ps_dq = psmB.tile([128, N], F32, name="ps_dq",
                                              tag="ps_dq", bufs=1)
                            us = {}
                            for hh in range(2):
                                h = 2 * i + hh
                                for kb in range(NT):
                                    Pk = NSZ[kb]
                                    ps = psmB.tile([128, N], F32, name="ps_st",
                                                   tag="ps_st", bufs=2)
                                    nc.tensor.matmul(
                                        ps[:Pk, :],
                                        kth[h][:, NOFF[kb]:NOFF[kb] + Pk],
                                        qth[h][:], start=True, stop=True)
                                    u = rp.tile([Pk, N], MMDT, name="u_st",
                                                tag="u_st")
                                    nc.scalar.activation(u[:], ps[:Pk, :],
                                                         AF.Exp,
                                                         bias=nclse[:Pk, :],
                                                         scale=1.0)
                                    us[(hh, kb)] = u
                            if i < HT_ - 1:
                                for mt in range(4 * i + 4, 4 * i + 8):
                                    dg_block(0, "hr", mt)
                            for hh in range(2):
                                h = 2 * i + hh
                                off = 64 * hh
                                for ns in range(NT):
                                    nc.tensor.matmul(
                                        ps_dk[off:off + 64, :],
                                        qp[ns][:, h * 64:(h + 1) * 64],
                                        e_h[h][ns][:], start=(ns == 0),
                                        stop=(ns == NT - 1))
                            nc.vector.tensor_copy(dkt_t[i][:], ps_dk[:])
                            for hh in range(2):
                                h = 2 * i + hh
                                off = 64 * hh
                                for kb in range(NT):
                                    nc.tensor.matmul(
                                        ps_dq[off:off + 64, :],
                                        kp[kb][:, h * 64:(h + 1) * 64],
                                        us[(hh, kb)][:], start=(kb == 0),
                                        stop=(kb == NT - 1))
                            nc.vector.tensor_copy(dqt_t[i][:], ps_dq[:])
                            if i > 0:
                                dg_block(0, "dkt", i - 1)
                                dg_block(0, "dqt", i - 1)
                        dg_block(0, "dkt", HT_ - 1)
                        dg_block(0, "dqt", HT_ - 1)
                        tail_ns(0)
                        for mt in range(MT_):
                            dg_block(1, "hr", mt)
                        for i in range(HT_):
                            dg_block(1, "dkt", i)
                            dg_block(1, "dqt", i)
                        tail_ns(1)

    nc.compile()
    return nc


def _prep_inputs(x, gamma, delta, wk, wq, xi):
    """Host-side weight transforms. Returns per-core in_maps."""
    npdt = _np_mmdt()
    gamma = np.asarray(gamma, np.float32)
    delta = np.asarray(delta, np.float32)
    Wk = np.asarray(wk, np.float32).reshape(HY, D)
    Wq = np.asarray(wq, np.float32).reshape(HY, D)
    Xi = np.asarray(xi, np.float32)

    Wks = Wk * gamma[None, :]
    Wqs = Wq * gamma[None, :]
    Xis = Xi * gamma[None, :]

    def rowblob(Wmat, nb):
        """[nb*128, D] row-major -> [128, nb*DW] with summed col appended."""
        ws = np.concatenate([Wmat, Wmat.sum(axis=1, keepdims=True)], axis=1)
        return ws.reshape(nb, 128, DW).transpose(1, 0, 2).reshape(128, nb * DW)

    # lhsT layouts [128, 4608] (d-major), beta folded into the K side
    wkt = (BETA * Wks).T.reshape(DT_, 128, HY).transpose(1, 0, 2).reshape(
        128, DT_ * HY)
    wqt = Wqs.T.reshape(DT_, 128, HY).transpose(1, 0, 2).reshape(128, DT_ * HY)
    wkr = rowblob(Wks, HT_)
    wqr = rowblob(Wqs / BETA, HT_)
    wblob = np.ascontiguousarray(
        np.concatenate([wkt, wqt, wkr, wqr], axis=1)).astype(npdt)

    # xit[mt][:, j*128:(j+1)*128] = Xis[mt-block, d-block j].T
    xit = np.concatenate(
        [Xis.reshape(MT_, 128, DT_, 128)[:, :, j, :].transpose(0, 2, 1)
         for j in range(DT_)], axis=2)
    xitb = np.ascontiguousarray(
        xit.transpose(1, 0, 2).reshape(128, MT_ * D)).astype(npdt)
    xirb = np.ascontiguousarray(rowblob(Xis, MT_)).astype(npdt)

    biasb = np.zeros((128, 36), np.float32)
    biasb[:, 0:6] = (BETA * (Wk @ delta)).reshape(HT_, 128).T
    biasb[:, 6:12] = (Wq @ delta).reshape(HT_, 128).T
    biasb[:, 12:36] = (Xi @ delta).reshape(MT_, 128).T

    x = np.asarray(x, np.float32)
    shared = dict(wblob=wblob, xitb=xitb, xirb=xirb, biasb=biasb,
                  nonce=np.zeros((1, _nonce_cols()), np.float32))
    maps = []
    for b in range(B):
        xp = np.zeros((128, 2 * D), np.float32)
        xp[:, :D] = x[b, :128]
        xp[:NSZ[1], D:] = x[b, 128:]
        maps.append(dict(x=xp, **shared))
    return maps


def _unpack_out(o):
    """[128, 2D] packed -> [N, D]."""
    full = np.empty((N, D), np.float32)
    full[:128] = o[:, :D]
    full[128:] = o[:NSZ[1], D:]
    return full


def kernel(x, gamma, delta, wk, wq, xi, _trace=False):
    if "nc" not in _CACHE:
        _CACHE["nc"] = build_program()
    nc = _CACHE["nc"]
    in_maps = _prep_inputs(x, gamma, delta, wk, wq, xi)
    res = bass_utils.run_bass_kernel_spmd(
        nc, in_maps, core_ids=list(range(NCORES)), trace=_trace)
    out = np.stack([_unpack_out(res.results[c]["out"]) for c in range(NCORES)])
    if _trace:
        _CACHE["last_results"] = res
    return out

